# revision 55
# baseline (speedup 1.0000x reference)
"""Trainium2 kernel for nn_CustomizedMoGPositionwiseFF (moe_routing).

Strategy (expert-parallel, per the sharding hint):
  - 32 (group, expert) FFN pairs are sharded across 8 NeuronCores (4 each).
  - Routing (group top-2 gate + per-group inner top-2 gate) is computed on
    host at call time; tokens are dispatched (gathered) per expert into the
    per-core shards -- data-dependent sharding, compiled into the NEFF.
  - Each core runs both FFN matmuls + relu for its 4 experts over the tokens
    routed to them, reading each expert weight exactly once (memory regime).
    Weights and activations are shipped as fp8 e4m3 and the matmuls run in
    DoubleRow perf mode (2 fp8 weights per PE cell, 256-deep contraction):
    ~2x the bf16 PE throughput and half the weight DMA traffic.  PSUM
    accumulation stays f32; relative error vs the f32 reference ~4e-3.
  - Host applies the cheap O(N*D) combine: iw/b2 scaling, scatter-add of the
    two expert contributions per (token, group), per-group post-layernorm,
    group top-2 mixture, and the outer residual.

Schedule: every input load rides the single Sync-engine HWDGE ring in
exact consumption order as per-slot slab transfers (HWDGE transfers execute
FIFO per issuing engine), so the slot-0 critical path gets full bandwidth
and later slots' weights are naturally paced behind it; the ~1MB of output
stores queue behind the last slab so they never steal stream bandwidth.
The PE is kept busy from its first available cycle with dummy matmuls so
the HAM clock gate (needs ~3.4us sustained activity) opens right as the
first real weights land instead of 8us later.  The previous slot's four L2
chains are spread through the next slot's eight L1 chains so the DVE/ACT
PSUM drains are evenly loaded and never gate the PE on PSUM-buffer reuse.
Output u^T is staged token-major [128, C, DT] (one dense ~2KB/partition
store per slot); the last slot is dt-major and ships dt-pairs immediately
to shorten the tail.
"""

import os
import numpy as np

# Model dims (hardcoded per the contract; match the reference problem)
B, T, D, H = 2, 1024, 512, 2048
G, E, GK, EK = 4, 8, 2, 2
EPS = 1e-5
N = B * T
P = 128
DT = D // P    # 4 d-tiles
HT = H // P    # 16 h-tiles
NCORES = 8
SLOTS = (G * E) // NCORES  # 4 experts per core
CAP_GRAN = 1               # capacity granularity (tokens)
WARMUP_MM = 15             # dummy matmuls to hold the PE busy pre-weights

_nc_cache = {}
LAST_RESULTS = None       # test harness can inspect (BassKernelResults)


def _ensure_ntff_hook():
    """Register antenv.axon_hooks with the ctypes NTFF profile hook if the
    container's antenv package lacks it (mirrors trn_agent_boot.trn_boot).
    Makes trace=True work; degrades to hook=None when the .so is absent."""
    try:
        from antenv.axon_hooks import get_axon_ntff_profile_hook  # noqa: F401
        return
    except ImportError:
        pass
    import sys
    import types
    import contextlib
    import ctypes

    mod = types.ModuleType("antenv.axon_hooks")
    _state = {"hook": None}

    def set_axon_ntff_profile_hook(h):
        _state["hook"] = h

    def get_axon_ntff_profile_hook():
        return _state["hook"]

    mod.set_axon_ntff_profile_hook = set_axon_ntff_profile_hook
    mod.get_axon_ntff_profile_hook = get_axon_ntff_profile_hook

    so_path = "/opt/axon/libaxon_pjrt.so"
    hook = None
    if os.path.exists(so_path):
        try:
            lib = ctypes.CDLL(so_path)
            if hasattr(lib, "axon_start_nrt_profile"):
                lib.axon_start_nrt_profile.argtypes = [
                    ctypes.POINTER(ctypes.c_int64), ctypes.c_size_t]
                lib.axon_start_nrt_profile.restype = ctypes.c_int64
                lib.axon_stop_nrt_profile.argtypes = [ctypes.c_char_p]
                lib.axon_stop_nrt_profile.restype = ctypes.c_int64

                @contextlib.contextmanager
                def _hook(output_dir, device_ids):
                    import jax
                    jax.devices()
                    if device_ids:
                        ids = (ctypes.c_int64 * len(device_ids))(*device_ids)
                        rc = lib.axon_start_nrt_profile(ids, len(device_ids))
                    else:
                        rc = lib.axon_start_nrt_profile(None, 0)
                    if rc != 0:
                        raise RuntimeError(f"axon_start_nrt_profile rc={rc}")
                    try:
                        yield
                    finally:
                        n = lib.axon_stop_nrt_profile(str(output_dir).encode())
                        print(f"ntff profile: {n} file(s) -> {output_dir}")

                hook = _hook
        except Exception:
            hook = None
    _state["hook"] = hook
    import antenv
    sys.modules["antenv.axon_hooks"] = mod
    antenv.axon_hooks = mod


def _round_up(x, m):
    return ((x + m - 1) // m) * m


def _routing(inp, ln_g, ln_b, wg_group, wg_inner):
    """Replicate the reference gating bit-for-bit on jax-cpu.

    Returns gi [N,GK] group ids, gsc [N,GK] group softmax, z [N,D] f32,
    eis/escs: per-group inner top-k ids/softmax ([N,EK] each).
    """
    import jax
    import jax.numpy as jnp

    cpu = jax.devices("cpu")[0]
    with jax.default_device(cpu):
        x = jnp.asarray(np.asarray(inp, np.float32)).reshape(-1, D)
        gl = x @ jnp.asarray(np.asarray(wg_group, np.float32))
        gv, gi = jax.lax.top_k(gl, GK)
        gsc = jax.nn.softmax(gv, axis=-1)
        m = jnp.mean(x, axis=-1, keepdims=True)
        xc = x - m
        v = jnp.mean(xc * xc, axis=-1, keepdims=True)
        z = xc * jax.lax.rsqrt(v + EPS) * jnp.asarray(np.asarray(ln_g, np.float32)) \
            + jnp.asarray(np.asarray(ln_b, np.float32))
        wgi = jnp.asarray(np.asarray(wg_inner, np.float32))
        eis, escs = [], []
        for g in range(G):
            l = z @ wgi[g]
            ev, ei = jax.lax.top_k(l, EK)
            esc = jax.nn.softmax(ev, axis=-1)
            eis.append(np.asarray(ei))
            escs.append(np.asarray(esc))
    return np.asarray(gi), np.asarray(gsc), np.asarray(z), eis, escs


def _build_nc(Cs, has_b1=False):
    """Build the SPMD Bass program for per-slot capacities Cs (uniform across cores).

    fp8 e4m3 weights + activations, DoubleRowSwInterleave matmuls (256-deep
    contraction; weights pre-interleaved on host so the stationary load is a
    dense 256-column read).
    """
    import concourse.bass as bass
    import concourse.bacc as bacc
    import concourse.tile as tile
    from concourse import mybir

    f32 = mybir.dt.float32
    bf16 = mybir.dt.bfloat16
    f8 = mybir.dt.float8e4
    DR = mybir.MatmulPerfMode.DoubleRowSwInterleave
    Relu = mybir.ActivationFunctionType.Relu
    Copy = mybir.ActivationFunctionType.Copy

    CT = int(sum(Cs))
    offs = np.concatenate([[0], np.cumsum(Cs)]).astype(int)
    CMAX = int(max(Cs))

    # per-partition byte layout of each slot's input slab:
    #   zt (DT*C fp8, d-major) || w1 (HT*2*256, ht-major SwInterleave blocks)
    #   || w2 (DT*8*256, dt-major SwInterleave blocks)
    W1B = HT * (DT // 2) * 2 * P          # 8192 B/partition
    W2B = DT * (HT // 2) * 2 * P          # 8192 B/partition
    W1_HEAD_HT = 6                        # slot-0 w1 h-tiles shipped with zt
    HB = 2                                # h-tiles per L1 PSUM group (each
                                          # matmul output must stay inside
                                          # one 2KB PSUM bank -> 512 stride)

    nc = bacc.Bacc("TRN2", target_bir_lowering=False)
    # DMA plan: the big cost is per-transfer overhead + ring serialization,
    # so each slot's entire input is ONE dense [128, bytes] slab transfer on
    # the Sync HWDGE ring (FIFO per engine => strict consumption order).
    # Slot 0 is split in three (zt+w1 head / w1 tail / w2) so the very first
    # matmuls start ~0.3MB in instead of 2.2MB in.
    C0 = int(Cs[0])
    slabA_d = nc.declare_dram_parameter(
        "slabA", [P, DT * C0 + 2 * (DT // 2) * 2 * P], f8, isOutput=False)
    slabA1_d = nc.declare_dram_parameter(
        "slabA1", [P, (W1_HEAD_HT - 2) * (DT // 2) * 2 * P], f8, isOutput=False)
    slabB_d = nc.declare_dram_parameter(
        "slabB", [P, (HT - W1_HEAD_HT) * (DT // 2) * 2 * P], f8, isOutput=False)
    slabC_d = nc.declare_dram_parameter("slabC", [P, W2B], f8, isOutput=False)
    # per-slot (s>=1) the slab is split in two: a = zt+w1 (gates the slot's
    # L1, fires ~2.5us earlier than a combined slab would), b = w2.
    slab_a_d = [None] + [
        nc.declare_dram_parameter(f"slab{s}a", [P, DT * int(Cs[s]) + W1B],
                                  f8, isOutput=False)
        for s in range(1, SLOTS)]
    slab_b_d = [None] + [
        nc.declare_dram_parameter(f"slab{s}b", [P, W2B], f8, isOutput=False)
        for s in range(1, SLOTS)]
    if has_b1:
        b1_d = nc.declare_dram_parameter("b1", [P, SLOTS * HT], f32, isOutput=False)
    # token-major output for slots 0..SLOTS-2: one dense per-partition line
    # per slot.  The last slot gets a dt-major tensor of its own so its
    # dt-pair stores are dense too (a [.., C, dt0:dt1] slice of a token-major
    # tensor would be a 4-byte-strided descriptor explosion).
    CT_HEAD = int(sum(Cs[:-1]))
    u_d = nc.declare_dram_parameter("u", [P, CT_HEAD, DT], bf16, isOutput=True)
    u2_d = nc.declare_dram_parameter("u2", [P, DT, int(Cs[-1])], bf16, isOutput=True)

    with tile.TileContext(nc) as tc:
        # PSUM budget (8 banks of 2KB/partition): hpsum 3x2 + upsum 2x1 = 8
        with tc.tile_pool(name="consts", bufs=1) as consts, \
             tc.tile_pool(name="hpool", bufs=2) as hpool, \
             tc.tile_pool(name="hpsum", bufs=3, space="PSUM") as hpsum, \
             tc.tile_pool(name="upsum", bufs=2, space="PSUM") as upsum, \
             tc.tile_pool(name="usb", bufs=3) as usb:

            slabA_sb = consts.tile(list(slabA_d.shape), f8, tag="slabA")
            slabA1_sb = consts.tile(list(slabA1_d.shape), f8, tag="slabA1")
            slabB_sb = consts.tile(list(slabB_d.shape), f8, tag="slabB")
            slabC_sb = consts.tile(list(slabC_d.shape), f8, tag="slabC")
            slab_a_sb = [None] + [consts.tile(list(slab_a_d[s].shape), f8,
                                              tag=f"slab{s}a", name=f"slab_a{s}")
                                  for s in range(1, SLOTS)]
            slab_b_sb = [None] + [consts.tile(list(slab_b_d[s].shape), f8,
                                              tag=f"slab{s}b", name=f"slab_b{s}")
                                  for s in range(1, SLOTS)]
            if has_b1:
                b1_sb = consts.tile([P, SLOTS * HT], f32, tag="b1")
            zero_sb = consts.tile([P, HB, CMAX], f32, tag="zero")
            dummy_sb = consts.tile([P, 512], f8, tag="dummy")
            dscr_sb = consts.tile([P, 4], f32, tag="dscr")
            # gpsimd is the earliest-ready data engine after the framework
            # preamble; it seeds the warm-up operand so the PE can start
            # dummy matmuls ~1us sooner than a DVE memset would allow.
            nc.gpsimd.memset(dummy_sb[:, :], 0.0)

            def ztv(s):
                C = int(Cs[s])
                t = slabA_sb if s == 0 else slab_a_sb[s]
                return t[:, :DT * C].rearrange("p (d c) -> p d c", d=DT)

            def w1v(s, ht, j):
                # one 256B SwInterleave stationary block, as [P, 2, 128]
                if s == 0:
                    if ht < 2:
                        t, base = slabA_sb, DT * C0
                    elif ht < W1_HEAD_HT:
                        t, base, ht = slabA1_sb, 0, ht - 2
                    else:
                        t, base, ht = slabB_sb, 0, ht - W1_HEAD_HT
                else:
                    t, base = slab_a_sb[s], DT * int(Cs[s])
                a = base + (ht * (DT // 2) + j) * 2 * P
                return t[:, a:a + 2 * P].rearrange("p (k m) -> p k m", k=2)

            def w2v(s, dt, t_):
                if s == 0:
                    t, base = slabC_sb, 0
                else:
                    t, base = slab_b_sb[s], 0
                a = base + (dt * (HT // 2) + t_) * 2 * P
                return t[:, a:a + 2 * P].rearrange("p (k m) -> p k m", k=2)

            # ---- PE warm-up: dummy matmuls with no input dependencies keep
            # the PE's HAM activity monitor busy from the engine's very first
            # available cycle, bridging gap-free into the real stream so the
            # 2.4 GHz clock gate (needs ~3.4us of sustained activity) opens
            # shortly after the real matmuls begin.
            phd = hpsum.tile([P, 2, 512], f32, tag="ph")
            for _ in range(WARMUP_MM):
                nc.tensor.matmul(
                    phd[:, 0, :256],
                    dummy_sb[:, :256].rearrange("p (k m) -> p k m", k=2),
                    dummy_sb[:, :].rearrange("p (k m) -> p k m", k=2),
                    start=True, stop=True, perf_mode=DR,
                )
            # tiny read of the warm PSUM so the matmuls can't be elided
            nc.vector.tensor_copy(dscr_sb[:, :], phd[:, 0, :4])

            # ---- resident loads: ALL on the Sync HWDGE ring, in exact
            # consumption order.  HWDGE executes one engine's transfers in
            # FIFO order, so each transfer gets the full SDMA bandwidth and
            # later slots can never starve the critical slot-0 path.
            # (measured: the Scalar HWDGE ring is slower for this first
            # transfer than queueing it first on Sync, so everything rides
            # the Sync ring)
            nc.sync.dma_start(slabA_sb[:, :], slabA_d[:, :])
            nc.sync.dma_start(slabA1_sb[:, :], slabA1_d[:, :])
            nc.sync.dma_start(slabB_sb[:, :], slabB_d[:, :])
            if has_b1:
                nc.sync.dma_start(b1_sb[:, :], b1_d[:, :])
            nc.sync.dma_start(slabC_sb[:, :W2B // 2], slabC_d[:, :W2B // 2])
            nc.sync.dma_start(slabC_sb[:, W2B // 2:], slabC_d[:, W2B // 2:])
            for s in range(1, SLOTS):
                nc.sync.dma_start(slab_a_sb[s][:, :], slab_a_d[s][:, :])
                nc.sync.dma_start(slab_b_sb[s][:, :], slab_b_d[s][:, :])

            # zeros for the DVE relu (max vs 0) path
            nc.vector.memset(zero_sb[:, :, :], 0.0)

            # ---- compute
            h_tiles, u_tiles = {}, {}

            def emit_l1_chunk(s, hb):
                # layer 1: h^T[ht] = relu(W1^T z^T + b1), 2 interleaved-fp8
                # matmuls (256-deep contraction each) per output tile.  Three
                # h-tiles share one 2-bank PSUM tile (3*C*4B <= 4KB) so the
                # relu drains them in a single op, alternating DVE / ACT
                # (GPSIMD can't read PSUM): fewer, bigger drains keep the
                # engines off the PE's critical path.
                C = int(Cs[s])
                if s not in h_tiles:
                    h_tiles[s] = hpool.tile([P, HT, C], f8, tag="h",
                                            name=f"h_{s}")
                h_sb = h_tiles[s]
                zt_ap = ztv(s)
                hts = range(HB * hb, min(HB * hb + HB, HT))
                n = len(hts)
                ph = hpsum.tile([P, HB, 512], f32, tag="ph")
                for i, ht in enumerate(hts):
                    for j in range(DT // 2):
                        nc.tensor.matmul(
                            ph[:, i, :C],
                            w1v(s, ht, j),
                            zt_ap[:, 2 * j:2 * j + 2, :],
                            start=(j == 0),
                            stop=(j == DT // 2 - 1),
                            perf_mode=DR,
                        )
                if has_b1:
                    # general path: ACT relu with per-partition bias
                    for i, ht in enumerate(hts):
                        nc.scalar.activation(
                            h_sb[:, ht, :], ph[:, i, :C], Relu,
                            bias=b1_sb[:, s * HT + ht: s * HT + ht + 1],
                        )
                elif hb % 2 == 0:
                    # DVE TT: relu(x) = max(x, 0) vs a zeros tile
                    nc.vector.tensor_max(
                        h_sb[:, HB * hb:HB * hb + n, :],
                        ph[:, :n, :C],
                        zero_sb[:, :n, :C])
                else:
                    nc.scalar.activation(
                        h_sb[:, HB * hb:HB * hb + n, :], ph[:, :n, :C], Relu)

            def emit_l2_chunk(s, dt):
                # layer 2: u^T[dt] = sum_ht W2[ht,dt]^T h^T[ht], 8 matmuls
                # per output tile.  Early slots stage token-major [C, DT];
                # the last slot stages dt-major so each drained dt-pair ships
                # immediately as a dense transfer.
                C = int(Cs[s])
                off = int(offs[s])
                last = s == SLOTS - 1
                if s not in u_tiles:
                    u_tiles[s] = usb.tile([P, DT, C] if last else [P, C, DT],
                                          bf16, tag="u2" if last else "u",
                                          name=f"u_{s}")
                u_sb = u_tiles[s]
                h_sb = h_tiles[s]
                pu = upsum.tile([P, CMAX], f32, tag="pu")
                for t in range(HT // 2):
                    nc.tensor.matmul(
                        pu[:, :C],
                        w2v(s, dt, t),
                        h_sb[:, 2 * t:2 * t + 2, :],
                        start=(t == 0),
                        stop=(t == HT // 2 - 1),
                        perf_mode=DR,
                    )
                u_ap = u_sb[:, dt, :] if last else u_sb[:, :, dt]
                if dt % 2 == 1:
                    nc.scalar.activation(u_ap, pu[:, :C], Copy)
                else:
                    nc.vector.tensor_copy(u_ap, pu[:, :C])
                if last and dt % 2 == 1:
                    # sync ring is idle by now; issuing there overlaps
                    # the store with the scalar engine's next ACT copy
                    nc.sync.dma_start(
                        u2_d[:, dt - 1:dt + 1, :],
                        u_sb[:, dt - 1:dt + 1, :])
                if not last and dt == DT - 1:
                    # earlier slots: one dense store on the Sync ring, queued
                    # BEHIND all weight slabs (FIFO) so the ~1MB of output
                    # writes never steal fabric bandwidth from the weight
                    # stream; they drain late when the ring is idle.
                    nc.sync.dma_start(u_d[:, off:off + C, :], u_sb[:, :, :])

            # Emission order: slot 0's L1 runs straight (supply-paced; the
            # drains hide under the incoming weight stream).  After that,
            # slot s-1's four L2 chains are spread evenly through slot s's
            # eight L1 chains: during an L1 burst the DVE/ACT relu drains
            # (~0.8us per 2-htile) can't keep up with the 0.51us PSUM fill
            # rate, so without the mix the PE stalls on PSUM-buffer reuse
            # while both engines idle during the following L2 burst.
            for hp in range(HT // 2):
                emit_l1_chunk(0, hp)
            for s in range(1, SLOTS):
                emit_l2_chunk(s - 1, 0)
                emit_l1_chunk(s, 0)
                emit_l1_chunk(s, 1)
                emit_l2_chunk(s - 1, 1)
                emit_l1_chunk(s, 2)
                emit_l1_chunk(s, 3)
                emit_l2_chunk(s - 1, 2)
                emit_l1_chunk(s, 4)
                emit_l1_chunk(s, 5)
                emit_l2_chunk(s - 1, 3)
                emit_l1_chunk(s, 6)
                emit_l1_chunk(s, 7)
            for dt in range(DT):
                emit_l2_chunk(SLOTS - 1, dt)
    nc.compile()
    return nc


def _get_nc(Cs, has_b1):
    key = (tuple(int(c) for c in Cs), bool(has_b1))
    if key not in _nc_cache:
        _nc_cache[key] = _build_nc(key[0], key[1])
    return _nc_cache[key]


def kernel(inp, ln_g, ln_b, wg_group, wg_inner, W1, b1, W2, b2, gln_g, gln_b):
    global LAST_RESULTS
    import jax
    import jax.numpy as jnp
    import ml_dtypes

    inp = np.asarray(inp)
    in_dtype = inp.dtype
    bf = ml_dtypes.bfloat16
    f8 = ml_dtypes.float8_e4m3

    # ---- 1. routing on host (bit-exact replica of the reference gates)
    gi, gsc, z, eis, escs = _routing(inp, ln_g, ln_b, wg_group, wg_inner)

    # token lists per (g, e)
    tok_lists, scale_lists = {}, {}
    for g in range(G):
        in_g = (gi == g).any(axis=1)
        S_g = np.nonzero(in_g)[0]
        ei, esc = eis[g], escs[g]
        for e in range(E):
            sel = ei[S_g] == e           # [|S_g|, EK]
            has = sel.any(axis=1)
            toks = S_g[has]
            w = (esc[S_g] * sel).sum(axis=1)[has]
            tok_lists[(g, e)] = toks
            scale_lists[(g, e)] = w.astype(np.float32)

    # ---- 2. balanced assignment of the 32 pairs to (core, slot)
    pairs = [(g, e) for g in range(G) for e in range(E)]
    pairs.sort(key=lambda p: -len(tok_lists[p]))
    assign = {}           # (core, slot) -> (g, e)
    Cs = []
    for s in range(SLOTS):
        rank = pairs[s * NCORES:(s + 1) * NCORES]
        Cs.append(max(CAP_GRAN, _round_up(max(len(tok_lists[p]) for p in rank), CAP_GRAN)))
        for c, p in enumerate(rank):
            assign[(c, s)] = p
    CT = int(sum(Cs))
    offs = np.concatenate([[0], np.cumsum(Cs)]).astype(int)

    # ---- 3. build per-core input maps
    W1n = np.asarray(W1, np.float32)
    W2n = np.asarray(W2, np.float32)
    b1n = np.asarray(b1, np.float32)
    b2n = np.asarray(b2, np.float32)
    z_f8 = z.astype(f8)
    has_b1 = bool(np.any(b1n))

    def _swi(W, n_in_tiles, n_out_tiles):
        # [K, M] weight -> the PE SwInterleave stationary layout
        # [ki, out_tile, pair, 256] with columns [A127 B127 .. A0 B0]
        # (pair-interleaved, out-column-reversed).
        Wv = W.astype(f8).reshape(n_in_tiles, P, n_out_tiles, P)  # [q, ki, ot, m]
        Wp = Wv.reshape(n_in_tiles // 2, 2, P, n_out_tiles, P)    # [pair, ab, ki, ot, m]
        Wr = Wp[..., ::-1]                                        # reverse m
        # -> [ki, ot, pair, m, ab] -> interleave (m, ab) into 256
        return np.ascontiguousarray(Wr.transpose(2, 3, 0, 4, 1)).reshape(
            P, n_out_tiles, n_in_tiles // 2, 2 * P)

    W1_HEAD_HT = 6
    in_maps = []
    for c in range(NCORES):
        im = {}
        if has_b1:
            b1_np = np.empty((P, SLOTS * HT), np.float32)
            b1_v = b1_np.reshape(P, SLOTS, HT)
            im["b1"] = b1_np
        for s in range(SLOTS):
            g, e = assign[(c, s)]
            toks = tok_lists[(g, e)]
            n = len(toks)
            # z^T tile (dt, p, c) -> [p, dt, c], one contiguous block per slot
            zt_np = np.zeros((P, DT, int(Cs[s])), f8)
            zt_np[:, :, :n] = z_f8[toks].T.reshape(DT, P, n).transpose(1, 0, 2)
            # [P, HT, DT//2, 256] / [P, DT, HT//2, 256] SwInterleave blocks
            w1_np = _swi(W1n[g, e], DT, HT).reshape(P, -1)
            w2_np = _swi(W2n[g, e], HT, DT).reshape(P, -1)
            zflat = zt_np.reshape(P, -1)
            w1_head_b = W1_HEAD_HT * (DT // 2) * 2 * P
            w1_a0_b = 2 * (DT // 2) * 2 * P
            if s == 0:
                im["slabA"] = np.ascontiguousarray(
                    np.concatenate([zflat, w1_np[:, :w1_a0_b]], axis=1))
                im["slabA1"] = np.ascontiguousarray(w1_np[:, w1_a0_b:w1_head_b])
                im["slabB"] = np.ascontiguousarray(w1_np[:, w1_head_b:])
                im["slabC"] = np.ascontiguousarray(w2_np)
            else:
                im[f"slab{s}a"] = np.ascontiguousarray(
                    np.concatenate([zflat, w1_np], axis=1))
                im[f"slab{s}b"] = np.ascontiguousarray(w2_np)
            if has_b1:
                b1_v[:, s, :] = b1n[g, e].reshape(HT, P).T
        in_maps.append(im)

    # ---- 4. compile + run on the 8 NeuronCores
    _ensure_ntff_hook()
    from concourse.bass_utils import run_bass_kernel_spmd

    nc = _get_nc(Cs, has_b1)
    res = run_bass_kernel_spmd(
        nc, in_maps, core_ids=list(range(NCORES)),
        trace=bool(int(os.environ.get("KERNEL_TRACE", "0"))),
    )
    LAST_RESULTS = res

    # ---- 5. host combine
    moe = np.zeros((G, N, D), np.float32)
    for c in range(NCORES):
        # u [p, CT_HEAD, dt] token-major + u2 [p, dt, C_last] dt-major
        # -> u^T[d, c] with d = dt*P + p -> [CT, D]
        u_head = (
            np.asarray(res.results[c]["u"], np.float32)
            .transpose(2, 0, 1).reshape(D, -1).T
        )
        u_last = (
            np.asarray(res.results[c]["u2"], np.float32)
            .transpose(1, 0, 2).reshape(D, -1).T
        )
        u = np.concatenate([u_head, u_last], axis=0)
        for s in range(SLOTS):
            g, e = assign[(c, s)]
            toks = tok_lists[(g, e)]
            n = len(toks)
            w = scale_lists[(g, e)]
            contrib = u[offs[s]:offs[s] + n] * w[:, None] + w[:, None] * b2n[g, e][None, :]
            np.add.at(moe[g], toks, contrib)

    cpu = jax.devices("cpu")[0]
    with jax.default_device(cpu):
        zj = jnp.asarray(z)
        gi_j = jnp.asarray(gi)
        gsc_j = jnp.asarray(gsc)
        gw_dense = jnp.sum(
            jax.nn.one_hot(gi_j, G, dtype=jnp.float32) * gsc_j[..., None], axis=-2
        )  # [N, G]
        out = jnp.zeros((N, D), jnp.float32)
        gg = jnp.asarray(np.asarray(gln_g, np.float32))
        gb = jnp.asarray(np.asarray(gln_b, np.float32))
        for g in range(G):
            t = zj + jnp.asarray(moe[g])
            m = jnp.mean(t, axis=-1, keepdims=True)
            tc_ = t - m
            v = jnp.mean(tc_ * tc_, axis=-1, keepdims=True)
            y = tc_ * jax.lax.rsqrt(v + EPS) * gg[g] + gb[g]
            out = out + gw_dense[:, g:g + 1] * y
        result = np.asarray(out).reshape(B, T, D) + np.asarray(inp, np.float32)

    return result.astype(in_dtype)


# revision 56
# speedup vs baseline: 1.0427x; 1.0427x over previous
"""Trainium2 kernel for nn_CustomizedMoGPositionwiseFF (moe_routing).

Strategy (expert-parallel, per the sharding hint):
  - 32 (group, expert) FFN pairs are sharded across 8 NeuronCores (4 each).
  - Routing (group top-2 gate + per-group inner top-2 gate) is computed on
    host at call time; tokens are dispatched (gathered) per expert into the
    per-core shards -- data-dependent sharding, compiled into the NEFF.
  - Each core runs both FFN matmuls + relu for its 4 experts over the tokens
    routed to them, reading each expert weight exactly once (memory regime).
    Weights and activations are shipped as fp8 e4m3 and the matmuls run in
    DoubleRow perf mode (2 fp8 weights per PE cell, 256-deep contraction):
    ~2x the bf16 PE throughput and half the weight DMA traffic.  PSUM
    accumulation stays f32; relative error vs the f32 reference ~4e-3.
  - Host applies the cheap O(N*D) combine: iw/b2 scaling, scatter-add of the
    two expert contributions per (token, group), per-group post-layernorm,
    group top-2 mixture, and the outer residual.

Schedule: every input load rides the single Sync-engine HWDGE ring in
exact consumption order as per-slot slab transfers (HWDGE transfers execute
FIFO per issuing engine), so the slot-0 critical path gets full bandwidth
and later slots' weights are naturally paced behind it; the ~1MB of output
stores queue behind the last slab so they never steal stream bandwidth.
The PE is kept busy from its first available cycle with dummy matmuls so
the HAM clock gate (needs ~3.4us sustained activity) opens right as the
first real weights land instead of 8us later.  The previous slot's four L2
chains are spread through the next slot's eight L1 chains so the DVE/ACT
PSUM drains are evenly loaded and never gate the PE on PSUM-buffer reuse.
Output u^T is staged token-major [128, C, DT] (one dense ~2KB/partition
store per slot); the last slot is dt-major and ships dt-pairs immediately
to shorten the tail.
"""

import os
import numpy as np

# Model dims (hardcoded per the contract; match the reference problem)
B, T, D, H = 2, 1024, 512, 2048
G, E, GK, EK = 4, 8, 2, 2
EPS = 1e-5
N = B * T
P = 128
DT = D // P    # 4 d-tiles
HT = H // P    # 16 h-tiles
NCORES = 8
SLOTS = (G * E) // NCORES  # 4 experts per core
CAP_GRAN = 1               # capacity granularity (tokens)
WARMUP_MM = 17             # dummy matmuls to hold the PE busy pre-weights

_nc_cache = {}
LAST_RESULTS = None       # test harness can inspect (BassKernelResults)


def _ensure_ntff_hook():
    """Register antenv.axon_hooks with the ctypes NTFF profile hook if the
    container's antenv package lacks it (mirrors trn_agent_boot.trn_boot).
    Makes trace=True work; degrades to hook=None when the .so is absent."""
    try:
        from antenv.axon_hooks import get_axon_ntff_profile_hook  # noqa: F401
        return
    except ImportError:
        pass
    import sys
    import types
    import contextlib
    import ctypes

    mod = types.ModuleType("antenv.axon_hooks")
    _state = {"hook": None}

    def set_axon_ntff_profile_hook(h):
        _state["hook"] = h

    def get_axon_ntff_profile_hook():
        return _state["hook"]

    mod.set_axon_ntff_profile_hook = set_axon_ntff_profile_hook
    mod.get_axon_ntff_profile_hook = get_axon_ntff_profile_hook

    so_path = "/opt/axon/libaxon_pjrt.so"
    hook = None
    if os.path.exists(so_path):
        try:
            lib = ctypes.CDLL(so_path)
            if hasattr(lib, "axon_start_nrt_profile"):
                lib.axon_start_nrt_profile.argtypes = [
                    ctypes.POINTER(ctypes.c_int64), ctypes.c_size_t]
                lib.axon_start_nrt_profile.restype = ctypes.c_int64
                lib.axon_stop_nrt_profile.argtypes = [ctypes.c_char_p]
                lib.axon_stop_nrt_profile.restype = ctypes.c_int64

                @contextlib.contextmanager
                def _hook(output_dir, device_ids):
                    import jax
                    jax.devices()
                    if device_ids:
                        ids = (ctypes.c_int64 * len(device_ids))(*device_ids)
                        rc = lib.axon_start_nrt_profile(ids, len(device_ids))
                    else:
                        rc = lib.axon_start_nrt_profile(None, 0)
                    if rc != 0:
                        raise RuntimeError(f"axon_start_nrt_profile rc={rc}")
                    try:
                        yield
                    finally:
                        n = lib.axon_stop_nrt_profile(str(output_dir).encode())
                        print(f"ntff profile: {n} file(s) -> {output_dir}")

                hook = _hook
        except Exception:
            hook = None
    _state["hook"] = hook
    import antenv
    sys.modules["antenv.axon_hooks"] = mod
    antenv.axon_hooks = mod


def _round_up(x, m):
    return ((x + m - 1) // m) * m


def _routing(inp, ln_g, ln_b, wg_group, wg_inner):
    """Replicate the reference gating bit-for-bit on jax-cpu.

    Returns gi [N,GK] group ids, gsc [N,GK] group softmax, z [N,D] f32,
    eis/escs: per-group inner top-k ids/softmax ([N,EK] each).
    """
    import jax
    import jax.numpy as jnp

    cpu = jax.devices("cpu")[0]
    with jax.default_device(cpu):
        x = jnp.asarray(np.asarray(inp, np.float32)).reshape(-1, D)
        gl = x @ jnp.asarray(np.asarray(wg_group, np.float32))
        gv, gi = jax.lax.top_k(gl, GK)
        gsc = jax.nn.softmax(gv, axis=-1)
        m = jnp.mean(x, axis=-1, keepdims=True)
        xc = x - m
        v = jnp.mean(xc * xc, axis=-1, keepdims=True)
        z = xc * jax.lax.rsqrt(v + EPS) * jnp.asarray(np.asarray(ln_g, np.float32)) \
            + jnp.asarray(np.asarray(ln_b, np.float32))
        wgi = jnp.asarray(np.asarray(wg_inner, np.float32))
        eis, escs = [], []
        for g in range(G):
            l = z @ wgi[g]
            ev, ei = jax.lax.top_k(l, EK)
            esc = jax.nn.softmax(ev, axis=-1)
            eis.append(np.asarray(ei))
            escs.append(np.asarray(esc))
    return np.asarray(gi), np.asarray(gsc), np.asarray(z), eis, escs


def _build_nc(Cs, has_b1=False):
    """Build the SPMD Bass program for per-slot capacities Cs (uniform across cores).

    fp8 e4m3 weights + activations, DoubleRowSwInterleave matmuls (256-deep
    contraction; weights pre-interleaved on host so the stationary load is a
    dense 256-column read).
    """
    import concourse.bass as bass
    import concourse.bacc as bacc
    import concourse.tile as tile
    from concourse import mybir

    f32 = mybir.dt.float32
    bf16 = mybir.dt.bfloat16
    f8 = mybir.dt.float8e4
    DR = mybir.MatmulPerfMode.DoubleRowSwInterleave
    Relu = mybir.ActivationFunctionType.Relu
    Copy = mybir.ActivationFunctionType.Copy

    CT = int(sum(Cs))
    offs = np.concatenate([[0], np.cumsum(Cs)]).astype(int)
    CMAX = int(max(Cs))

    # per-partition byte layout of each slot's input slab:
    #   zt (DT*C fp8, d-major) || w1 (HT*2*256, ht-major SwInterleave blocks)
    #   || w2 (DT*8*256, dt-major SwInterleave blocks)
    W1B = HT * (DT // 2) * 2 * P          # 8192 B/partition
    W2B = DT * (HT // 2) * 2 * P          # 8192 B/partition
    W1_HEAD_HT = 6                        # slot-0 w1 h-tiles shipped with zt
    HB = 2                                # h-tiles per L1 PSUM group (each
                                          # matmul output must stay inside
                                          # one 2KB PSUM bank -> 512 stride)

    nc = bacc.Bacc("TRN2", target_bir_lowering=False)
    # DMA plan: the big cost is per-transfer overhead + ring serialization,
    # so each slot's entire input is ONE dense [128, bytes] slab transfer on
    # the Sync HWDGE ring (FIFO per engine => strict consumption order).
    # Slot 0 is split in three (zt+w1 head / w1 tail / w2) so the very first
    # matmuls start ~0.3MB in instead of 2.2MB in.
    C0 = int(Cs[0])
    slabA_d = nc.declare_dram_parameter(
        "slabA", [P, DT * C0 + W1_HEAD_HT * (DT // 2) * 2 * P], f8, isOutput=False)
    slabB_d = nc.declare_dram_parameter(
        "slabB", [P, (HT - W1_HEAD_HT) * (DT // 2) * 2 * P], f8, isOutput=False)
    slabC_d = nc.declare_dram_parameter("slabC", [P, W2B], f8, isOutput=False)
    # per-slot (s>=1) the slab is split in two: a = zt+w1 (gates the slot's
    # L1, fires ~2.5us earlier than a combined slab would), b = w2.
    slab_a_d = [None] + [
        nc.declare_dram_parameter(f"slab{s}a", [P, DT * int(Cs[s]) + W1B],
                                  f8, isOutput=False)
        for s in range(1, SLOTS)]
    slab_b_d = [None] + [
        nc.declare_dram_parameter(f"slab{s}b", [P, W2B], f8, isOutput=False)
        for s in range(1, SLOTS)]
    if has_b1:
        b1_d = nc.declare_dram_parameter("b1", [P, SLOTS * HT], f32, isOutput=False)
    # token-major output for slots 0..SLOTS-2: one dense per-partition line
    # per slot.  The last slot gets a dt-major tensor of its own so its
    # dt-pair stores are dense too (a [.., C, dt0:dt1] slice of a token-major
    # tensor would be a 4-byte-strided descriptor explosion).
    CT_HEAD = int(sum(Cs[:-1]))
    u_d = nc.declare_dram_parameter("u", [P, CT_HEAD, DT], bf16, isOutput=True)
    u2_d = nc.declare_dram_parameter("u2", [P, DT, int(Cs[-1])], bf16, isOutput=True)

    with tile.TileContext(nc) as tc:
        # PSUM budget (8 banks of 2KB/partition): hpsum 3x2 + upsum 2x1 = 8
        with tc.tile_pool(name="consts", bufs=1) as consts, \
             tc.tile_pool(name="hpool", bufs=2) as hpool, \
             tc.tile_pool(name="hpsum", bufs=3, space="PSUM") as hpsum, \
             tc.tile_pool(name="upsum", bufs=2, space="PSUM") as upsum, \
             tc.tile_pool(name="usb", bufs=3) as usb:

            slabA_sb = consts.tile(list(slabA_d.shape), f8, tag="slabA")
            slabB_sb = consts.tile(list(slabB_d.shape), f8, tag="slabB")
            slabC_sb = consts.tile(list(slabC_d.shape), f8, tag="slabC")
            slab_a_sb = [None] + [consts.tile(list(slab_a_d[s].shape), f8,
                                              tag=f"slab{s}a", name=f"slab_a{s}")
                                  for s in range(1, SLOTS)]
            slab_b_sb = [None] + [consts.tile(list(slab_b_d[s].shape), f8,
                                              tag=f"slab{s}b", name=f"slab_b{s}")
                                  for s in range(1, SLOTS)]
            if has_b1:
                b1_sb = consts.tile([P, SLOTS * HT], f32, tag="b1")
            zero_sb = consts.tile([P, HB, CMAX], f32, tag="zero")
            dummy_sb = consts.tile([P, 512], f8, tag="dummy")
            dscr_sb = consts.tile([P, 4], f32, tag="dscr")
            # gpsimd is the earliest-ready data engine after the framework
            # preamble; it seeds the warm-up operand so the PE can start
            # dummy matmuls ~1us sooner than a DVE memset would allow.
            nc.gpsimd.memset(dummy_sb[:, :], 0.0)

            def ztv(s):
                C = int(Cs[s])
                t = slabA_sb if s == 0 else slab_a_sb[s]
                return t[:, :DT * C].rearrange("p (d c) -> p d c", d=DT)

            def w1v(s, ht, j):
                # one 256B SwInterleave stationary block, as [P, 2, 128]
                if s == 0:
                    if ht < W1_HEAD_HT:
                        t, base = slabA_sb, DT * C0
                    else:
                        t, base, ht = slabB_sb, 0, ht - W1_HEAD_HT
                else:
                    t, base = slab_a_sb[s], DT * int(Cs[s])
                a = base + (ht * (DT // 2) + j) * 2 * P
                return t[:, a:a + 2 * P].rearrange("p (k m) -> p k m", k=2)

            def w2v(s, dt, t_):
                if s == 0:
                    t, base = slabC_sb, 0
                else:
                    t, base = slab_b_sb[s], 0
                a = base + (dt * (HT // 2) + t_) * 2 * P
                return t[:, a:a + 2 * P].rearrange("p (k m) -> p k m", k=2)

            # ---- PE warm-up: dummy matmuls with no input dependencies keep
            # the PE's HAM activity monitor busy from the engine's very first
            # available cycle, bridging gap-free into the real stream so the
            # 2.4 GHz clock gate (needs ~3.4us of sustained activity) opens
            # shortly after the real matmuls begin.
            phd = hpsum.tile([P, 2, 512], f32, tag="ph")
            for _ in range(WARMUP_MM):
                nc.tensor.matmul(
                    phd[:, 0, :256],
                    dummy_sb[:, :256].rearrange("p (k m) -> p k m", k=2),
                    dummy_sb[:, :].rearrange("p (k m) -> p k m", k=2),
                    start=True, stop=True, perf_mode=DR,
                )
            # tiny read of the warm PSUM so the matmuls can't be elided
            nc.vector.tensor_copy(dscr_sb[:, :], phd[:, 0, :4])

            # ---- resident loads: ALL on the Sync HWDGE ring, in exact
            # consumption order.  HWDGE executes one engine's transfers in
            # FIFO order, so each transfer gets the full SDMA bandwidth and
            # later slots can never starve the critical slot-0 path.
            # (measured: the Scalar HWDGE ring is slower for this first
            # transfer than queueing it first on Sync, so everything rides
            # the Sync ring)
            nc.sync.dma_start(slabA_sb[:, :], slabA_d[:, :])
            nc.sync.dma_start(slabB_sb[:, :], slabB_d[:, :])
            if has_b1:
                nc.sync.dma_start(b1_sb[:, :], b1_d[:, :])
            nc.sync.dma_start(slabC_sb[:, :W2B // 2], slabC_d[:, :W2B // 2])
            nc.sync.dma_start(slabC_sb[:, W2B // 2:], slabC_d[:, W2B // 2:])
            for s in range(1, SLOTS):
                nc.sync.dma_start(slab_a_sb[s][:, :], slab_a_d[s][:, :])
                nc.sync.dma_start(slab_b_sb[s][:, :], slab_b_d[s][:, :])

            # zeros for the DVE relu (max vs 0) path
            nc.vector.memset(zero_sb[:, :, :], 0.0)

            # ---- compute
            h_tiles, u_tiles = {}, {}

            def emit_l1_chunk(s, hb):
                # layer 1: h^T[ht] = relu(W1^T z^T + b1), 2 interleaved-fp8
                # matmuls (256-deep contraction each) per output tile.  Three
                # h-tiles share one 2-bank PSUM tile (3*C*4B <= 4KB) so the
                # relu drains them in a single op, alternating DVE / ACT
                # (GPSIMD can't read PSUM): fewer, bigger drains keep the
                # engines off the PE's critical path.
                C = int(Cs[s])
                if s not in h_tiles:
                    h_tiles[s] = hpool.tile([P, HT, C], f8, tag="h",
                                            name=f"h_{s}")
                h_sb = h_tiles[s]
                zt_ap = ztv(s)
                hts = range(HB * hb, min(HB * hb + HB, HT))
                n = len(hts)
                ph = hpsum.tile([P, HB, 512], f32, tag="ph")
                for i, ht in enumerate(hts):
                    for j in range(DT // 2):
                        nc.tensor.matmul(
                            ph[:, i, :C],
                            w1v(s, ht, j),
                            zt_ap[:, 2 * j:2 * j + 2, :],
                            start=(j == 0),
                            stop=(j == DT // 2 - 1),
                            perf_mode=DR,
                        )
                if has_b1:
                    # general path: ACT relu with per-partition bias
                    for i, ht in enumerate(hts):
                        nc.scalar.activation(
                            h_sb[:, ht, :], ph[:, i, :C], Relu,
                            bias=b1_sb[:, s * HT + ht: s * HT + ht + 1],
                        )
                elif hb % 2 == 0:
                    # DVE TT: relu(x) = max(x, 0) vs a zeros tile
                    nc.vector.tensor_max(
                        h_sb[:, HB * hb:HB * hb + n, :],
                        ph[:, :n, :C],
                        zero_sb[:, :n, :C])
                else:
                    nc.scalar.activation(
                        h_sb[:, HB * hb:HB * hb + n, :], ph[:, :n, :C], Relu)

            def emit_l2_chunk(s, dt):
                # layer 2: u^T[dt] = sum_ht W2[ht,dt]^T h^T[ht], 8 matmuls
                # per output tile.  Early slots stage token-major [C, DT];
                # the last slot stages dt-major so each drained dt-pair ships
                # immediately as a dense transfer.
                C = int(Cs[s])
                off = int(offs[s])
                last = s == SLOTS - 1
                if s not in u_tiles:
                    u_tiles[s] = usb.tile([P, DT, C] if last else [P, C, DT],
                                          bf16, tag="u2" if last else "u",
                                          name=f"u_{s}")
                u_sb = u_tiles[s]
                h_sb = h_tiles[s]
                pu = upsum.tile([P, CMAX], f32, tag="pu")
                for t in range(HT // 2):
                    nc.tensor.matmul(
                        pu[:, :C],
                        w2v(s, dt, t),
                        h_sb[:, 2 * t:2 * t + 2, :],
                        start=(t == 0),
                        stop=(t == HT // 2 - 1),
                        perf_mode=DR,
                    )
                u_ap = u_sb[:, dt, :] if last else u_sb[:, :, dt]
                if dt % 2 == 1:
                    nc.scalar.activation(u_ap, pu[:, :C], Copy)
                else:
                    nc.vector.tensor_copy(u_ap, pu[:, :C])
                if last and dt % 2 == 1:
                    # sync ring is idle by now; issuing there overlaps
                    # the store with the scalar engine's next ACT copy
                    nc.sync.dma_start(
                        u2_d[:, dt - 1:dt + 1, :],
                        u_sb[:, dt - 1:dt + 1, :])
                if not last and dt == DT - 1:
                    # earlier slots: one dense store on the Sync ring, queued
                    # BEHIND all weight slabs (FIFO) so the ~1MB of output
                    # writes never steal fabric bandwidth from the weight
                    # stream; they drain late when the ring is idle.
                    nc.sync.dma_start(u_d[:, off:off + C, :], u_sb[:, :, :])

            # Emission order: slot 0's L1 runs straight (supply-paced; the
            # drains hide under the incoming weight stream).  After that,
            # slot s-1's four L2 chains are spread evenly through slot s's
            # eight L1 chains: during an L1 burst the DVE/ACT relu drains
            # (~0.8us per 2-htile) can't keep up with the 0.51us PSUM fill
            # rate, so without the mix the PE stalls on PSUM-buffer reuse
            # while both engines idle during the following L2 burst.
            for hp in range(HT // 2):
                emit_l1_chunk(0, hp)
            for s in range(1, SLOTS):
                emit_l2_chunk(s - 1, 0)
                emit_l1_chunk(s, 0)
                emit_l1_chunk(s, 1)
                emit_l2_chunk(s - 1, 1)
                emit_l1_chunk(s, 2)
                emit_l1_chunk(s, 3)
                emit_l2_chunk(s - 1, 2)
                emit_l1_chunk(s, 4)
                emit_l1_chunk(s, 5)
                emit_l2_chunk(s - 1, 3)
                emit_l1_chunk(s, 6)
                emit_l1_chunk(s, 7)
            for dt in range(DT):
                emit_l2_chunk(SLOTS - 1, dt)
    nc.compile()
    return nc


def _get_nc(Cs, has_b1):
    key = (tuple(int(c) for c in Cs), bool(has_b1))
    if key not in _nc_cache:
        _nc_cache[key] = _build_nc(key[0], key[1])
    return _nc_cache[key]


def kernel(inp, ln_g, ln_b, wg_group, wg_inner, W1, b1, W2, b2, gln_g, gln_b):
    global LAST_RESULTS
    import jax
    import jax.numpy as jnp
    import ml_dtypes

    inp = np.asarray(inp)
    in_dtype = inp.dtype
    bf = ml_dtypes.bfloat16
    f8 = ml_dtypes.float8_e4m3

    # ---- 1. routing on host (bit-exact replica of the reference gates)
    gi, gsc, z, eis, escs = _routing(inp, ln_g, ln_b, wg_group, wg_inner)

    # token lists per (g, e)
    tok_lists, scale_lists = {}, {}
    for g in range(G):
        in_g = (gi == g).any(axis=1)
        S_g = np.nonzero(in_g)[0]
        ei, esc = eis[g], escs[g]
        for e in range(E):
            sel = ei[S_g] == e           # [|S_g|, EK]
            has = sel.any(axis=1)
            toks = S_g[has]
            w = (esc[S_g] * sel).sum(axis=1)[has]
            tok_lists[(g, e)] = toks
            scale_lists[(g, e)] = w.astype(np.float32)

    # ---- 2. balanced assignment of the 32 pairs to (core, slot)
    pairs = [(g, e) for g in range(G) for e in range(E)]
    pairs.sort(key=lambda p: -len(tok_lists[p]))
    assign = {}           # (core, slot) -> (g, e)
    Cs = []
    for s in range(SLOTS):
        rank = pairs[s * NCORES:(s + 1) * NCORES]
        Cs.append(max(CAP_GRAN, _round_up(max(len(tok_lists[p]) for p in rank), CAP_GRAN)))
        for c, p in enumerate(rank):
            assign[(c, s)] = p
    CT = int(sum(Cs))
    offs = np.concatenate([[0], np.cumsum(Cs)]).astype(int)

    # ---- 3. build per-core input maps
    W1n = np.asarray(W1, np.float32)
    W2n = np.asarray(W2, np.float32)
    b1n = np.asarray(b1, np.float32)
    b2n = np.asarray(b2, np.float32)
    z_f8 = z.astype(f8)
    has_b1 = bool(np.any(b1n))

    def _swi(W, n_in_tiles, n_out_tiles):
        # [K, M] weight -> the PE SwInterleave stationary layout
        # [ki, out_tile, pair, 256] with columns [A127 B127 .. A0 B0]
        # (pair-interleaved, out-column-reversed).
        Wv = W.astype(f8).reshape(n_in_tiles, P, n_out_tiles, P)  # [q, ki, ot, m]
        Wp = Wv.reshape(n_in_tiles // 2, 2, P, n_out_tiles, P)    # [pair, ab, ki, ot, m]
        Wr = Wp[..., ::-1]                                        # reverse m
        # -> [ki, ot, pair, m, ab] -> interleave (m, ab) into 256
        return np.ascontiguousarray(Wr.transpose(2, 3, 0, 4, 1)).reshape(
            P, n_out_tiles, n_in_tiles // 2, 2 * P)

    W1_HEAD_HT = 6
    in_maps = []
    for c in range(NCORES):
        im = {}
        if has_b1:
            b1_np = np.empty((P, SLOTS * HT), np.float32)
            b1_v = b1_np.reshape(P, SLOTS, HT)
            im["b1"] = b1_np
        for s in range(SLOTS):
            g, e = assign[(c, s)]
            toks = tok_lists[(g, e)]
            n = len(toks)
            # z^T tile (dt, p, c) -> [p, dt, c], one contiguous block per slot
            zt_np = np.zeros((P, DT, int(Cs[s])), f8)
            zt_np[:, :, :n] = z_f8[toks].T.reshape(DT, P, n).transpose(1, 0, 2)
            # [P, HT, DT//2, 256] / [P, DT, HT//2, 256] SwInterleave blocks
            w1_np = _swi(W1n[g, e], DT, HT).reshape(P, -1)
            w2_np = _swi(W2n[g, e], HT, DT).reshape(P, -1)
            zflat = zt_np.reshape(P, -1)
            w1_head_b = W1_HEAD_HT * (DT // 2) * 2 * P
            if s == 0:
                im["slabA"] = np.ascontiguousarray(
                    np.concatenate([zflat, w1_np[:, :w1_head_b]], axis=1))
                im["slabB"] = np.ascontiguousarray(w1_np[:, w1_head_b:])
                im["slabC"] = np.ascontiguousarray(w2_np)
            else:
                im[f"slab{s}a"] = np.ascontiguousarray(
                    np.concatenate([zflat, w1_np], axis=1))
                im[f"slab{s}b"] = np.ascontiguousarray(w2_np)
            if has_b1:
                b1_v[:, s, :] = b1n[g, e].reshape(HT, P).T
        in_maps.append(im)

    # ---- 4. compile + run on the 8 NeuronCores
    _ensure_ntff_hook()
    from concourse.bass_utils import run_bass_kernel_spmd

    nc = _get_nc(Cs, has_b1)
    res = run_bass_kernel_spmd(
        nc, in_maps, core_ids=list(range(NCORES)),
        trace=bool(int(os.environ.get("KERNEL_TRACE", "0"))),
    )
    LAST_RESULTS = res

    # ---- 5. host combine
    moe = np.zeros((G, N, D), np.float32)
    for c in range(NCORES):
        # u [p, CT_HEAD, dt] token-major + u2 [p, dt, C_last] dt-major
        # -> u^T[d, c] with d = dt*P + p -> [CT, D]
        u_head = (
            np.asarray(res.results[c]["u"], np.float32)
            .transpose(2, 0, 1).reshape(D, -1).T
        )
        u_last = (
            np.asarray(res.results[c]["u2"], np.float32)
            .transpose(1, 0, 2).reshape(D, -1).T
        )
        u = np.concatenate([u_head, u_last], axis=0)
        for s in range(SLOTS):
            g, e = assign[(c, s)]
            toks = tok_lists[(g, e)]
            n = len(toks)
            w = scale_lists[(g, e)]
            contrib = u[offs[s]:offs[s] + n] * w[:, None] + w[:, None] * b2n[g, e][None, :]
            np.add.at(moe[g], toks, contrib)

    cpu = jax.devices("cpu")[0]
    with jax.default_device(cpu):
        zj = jnp.asarray(z)
        gi_j = jnp.asarray(gi)
        gsc_j = jnp.asarray(gsc)
        gw_dense = jnp.sum(
            jax.nn.one_hot(gi_j, G, dtype=jnp.float32) * gsc_j[..., None], axis=-2
        )  # [N, G]
        out = jnp.zeros((N, D), jnp.float32)
        gg = jnp.asarray(np.asarray(gln_g, np.float32))
        gb = jnp.asarray(np.asarray(gln_b, np.float32))
        for g in range(G):
            t = zj + jnp.asarray(moe[g])
            m = jnp.mean(t, axis=-1, keepdims=True)
            tc_ = t - m
            v = jnp.mean(tc_ * tc_, axis=-1, keepdims=True)
            y = tc_ * jax.lax.rsqrt(v + EPS) * gg[g] + gb[g]
            out = out + gw_dense[:, g:g + 1] * y
        result = np.asarray(out).reshape(B, T, D) + np.asarray(inp, np.float32)

    return result.astype(in_dtype)


# revision 57
# speedup vs baseline: 1.0653x; 1.0217x over previous
"""Trainium2 kernel for nn_CustomizedMoGPositionwiseFF (moe_routing).

Strategy (expert-parallel, per the sharding hint):
  - 32 (group, expert) FFN pairs are sharded across 8 NeuronCores (4 each).
  - Routing (group top-2 gate + per-group inner top-2 gate) is computed on
    host at call time; tokens are dispatched (gathered) per expert into the
    per-core shards -- data-dependent sharding, compiled into the NEFF.
  - Each core runs both FFN matmuls + relu for its 4 experts over the tokens
    routed to them, reading each expert weight exactly once (memory regime).
    Weights and activations are shipped as fp8 e4m3 and the matmuls run in
    DoubleRow perf mode (2 fp8 weights per PE cell, 256-deep contraction):
    ~2x the bf16 PE throughput and half the weight DMA traffic.  PSUM
    accumulation stays f32; relative error vs the f32 reference ~4e-3.
  - Host applies the cheap O(N*D) combine: iw/b2 scaling, scatter-add of the
    two expert contributions per (token, group), per-group post-layernorm,
    group top-2 mixture, and the outer residual.

Schedule: every input load rides the single Sync-engine HWDGE ring in
exact consumption order as per-slot slab transfers (HWDGE transfers execute
FIFO per issuing engine), so the slot-0 critical path gets full bandwidth
and later slots' weights are naturally paced behind it; the ~1MB of output
stores queue behind the last slab so they never steal stream bandwidth.
The PE is kept busy from its first available cycle with dummy matmuls so
the HAM clock gate (needs ~3.4us sustained activity) opens right as the
first real weights land instead of 8us later.  The previous slot's four L2
chains are spread through the next slot's eight L1 chains so the DVE/ACT
PSUM drains are evenly loaded and never gate the PE on PSUM-buffer reuse.
Output u^T is staged token-major [128, C, DT] (one dense ~2KB/partition
store per slot); the last slot is dt-major and ships dt-pairs immediately
to shorten the tail.
"""

import os
import numpy as np

# Model dims (hardcoded per the contract; match the reference problem)
B, T, D, H = 2, 1024, 512, 2048
G, E, GK, EK = 4, 8, 2, 2
EPS = 1e-5
N = B * T
P = 128
DT = D // P    # 4 d-tiles
HT = H // P    # 16 h-tiles
NCORES = 8
SLOTS = (G * E) // NCORES  # 4 experts per core
CAP_GRAN = 8               # capacity granularity (tokens; keeps every
                           # per-partition DMA line 32B-aligned)
WARMUP_MM = 17             # dummy matmuls to hold the PE busy pre-weights

_nc_cache = {}
LAST_RESULTS = None       # test harness can inspect (BassKernelResults)


def _ensure_ntff_hook():
    """Register antenv.axon_hooks with the ctypes NTFF profile hook if the
    container's antenv package lacks it (mirrors trn_agent_boot.trn_boot).
    Makes trace=True work; degrades to hook=None when the .so is absent."""
    try:
        from antenv.axon_hooks import get_axon_ntff_profile_hook  # noqa: F401
        return
    except ImportError:
        pass
    import sys
    import types
    import contextlib
    import ctypes

    mod = types.ModuleType("antenv.axon_hooks")
    _state = {"hook": None}

    def set_axon_ntff_profile_hook(h):
        _state["hook"] = h

    def get_axon_ntff_profile_hook():
        return _state["hook"]

    mod.set_axon_ntff_profile_hook = set_axon_ntff_profile_hook
    mod.get_axon_ntff_profile_hook = get_axon_ntff_profile_hook

    so_path = "/opt/axon/libaxon_pjrt.so"
    hook = None
    if os.path.exists(so_path):
        try:
            lib = ctypes.CDLL(so_path)
            if hasattr(lib, "axon_start_nrt_profile"):
                lib.axon_start_nrt_profile.argtypes = [
                    ctypes.POINTER(ctypes.c_int64), ctypes.c_size_t]
                lib.axon_start_nrt_profile.restype = ctypes.c_int64
                lib.axon_stop_nrt_profile.argtypes = [ctypes.c_char_p]
                lib.axon_stop_nrt_profile.restype = ctypes.c_int64

                @contextlib.contextmanager
                def _hook(output_dir, device_ids):
                    import jax
                    jax.devices()
                    if device_ids:
                        ids = (ctypes.c_int64 * len(device_ids))(*device_ids)
                        rc = lib.axon_start_nrt_profile(ids, len(device_ids))
                    else:
                        rc = lib.axon_start_nrt_profile(None, 0)
                    if rc != 0:
                        raise RuntimeError(f"axon_start_nrt_profile rc={rc}")
                    try:
                        yield
                    finally:
                        n = lib.axon_stop_nrt_profile(str(output_dir).encode())
                        print(f"ntff profile: {n} file(s) -> {output_dir}")

                hook = _hook
        except Exception:
            hook = None
    _state["hook"] = hook
    import antenv
    sys.modules["antenv.axon_hooks"] = mod
    antenv.axon_hooks = mod


def _round_up(x, m):
    return ((x + m - 1) // m) * m


def _routing(inp, ln_g, ln_b, wg_group, wg_inner):
    """Replicate the reference gating bit-for-bit on jax-cpu.

    Returns gi [N,GK] group ids, gsc [N,GK] group softmax, z [N,D] f32,
    eis/escs: per-group inner top-k ids/softmax ([N,EK] each).
    """
    import jax
    import jax.numpy as jnp

    cpu = jax.devices("cpu")[0]
    with jax.default_device(cpu):
        x = jnp.asarray(np.asarray(inp, np.float32)).reshape(-1, D)
        gl = x @ jnp.asarray(np.asarray(wg_group, np.float32))
        gv, gi = jax.lax.top_k(gl, GK)
        gsc = jax.nn.softmax(gv, axis=-1)
        m = jnp.mean(x, axis=-1, keepdims=True)
        xc = x - m
        v = jnp.mean(xc * xc, axis=-1, keepdims=True)
        z = xc * jax.lax.rsqrt(v + EPS) * jnp.asarray(np.asarray(ln_g, np.float32)) \
            + jnp.asarray(np.asarray(ln_b, np.float32))
        wgi = jnp.asarray(np.asarray(wg_inner, np.float32))
        eis, escs = [], []
        for g in range(G):
            l = z @ wgi[g]
            ev, ei = jax.lax.top_k(l, EK)
            esc = jax.nn.softmax(ev, axis=-1)
            eis.append(np.asarray(ei))
            escs.append(np.asarray(esc))
    return np.asarray(gi), np.asarray(gsc), np.asarray(z), eis, escs


def _build_nc(Cs, has_b1=False):
    """Build the SPMD Bass program for per-slot capacities Cs (uniform across cores).

    fp8 e4m3 weights + activations, DoubleRowSwInterleave matmuls (256-deep
    contraction; weights pre-interleaved on host so the stationary load is a
    dense 256-column read).
    """
    import concourse.bass as bass
    import concourse.bacc as bacc
    import concourse.tile as tile
    from concourse import mybir

    f32 = mybir.dt.float32
    bf16 = mybir.dt.bfloat16
    f8 = mybir.dt.float8e4
    DR = mybir.MatmulPerfMode.DoubleRowSwInterleave
    Relu = mybir.ActivationFunctionType.Relu
    Copy = mybir.ActivationFunctionType.Copy

    CT = int(sum(Cs))
    offs = np.concatenate([[0], np.cumsum(Cs)]).astype(int)
    CMAX = int(max(Cs))

    # per-partition byte layout of each slot's input slab:
    #   zt (DT*C fp8, d-major) || w1 (HT*2*256, ht-major SwInterleave blocks)
    #   || w2 (DT*8*256, dt-major SwInterleave blocks)
    W1B = HT * (DT // 2) * 2 * P          # 8192 B/partition
    W2B = DT * (HT // 2) * 2 * P          # 8192 B/partition
    W1_HEAD_HT = 6                        # slot-0 w1 h-tiles shipped with zt
    HB = 2                                # h-tiles per L1 PSUM group (each
                                          # matmul output must stay inside
                                          # one 2KB PSUM bank -> 512 stride)

    nc = bacc.Bacc("TRN2", target_bir_lowering=False)
    # DMA plan: the big cost is per-transfer overhead + ring serialization,
    # so each slot's entire input is ONE dense [128, bytes] slab transfer on
    # the Sync HWDGE ring (FIFO per engine => strict consumption order).
    # Slot 0 is split in three (zt+w1 head / w1 tail / w2) so the very first
    # matmuls start ~0.3MB in instead of 2.2MB in.
    C0 = int(Cs[0])
    slabA_d = nc.declare_dram_parameter(
        "slabA", [P, DT * C0 + W1_HEAD_HT * (DT // 2) * 2 * P], f8, isOutput=False)
    slabB_d = nc.declare_dram_parameter(
        "slabB", [P, (HT - W1_HEAD_HT) * (DT // 2) * 2 * P], f8, isOutput=False)
    slabC_d = nc.declare_dram_parameter("slabC", [P, W2B], f8, isOutput=False)
    # per-slot (s>=1) the slab is split in two: a = zt+w1 (gates the slot's
    # L1, fires ~2.5us earlier than a combined slab would), b = w2.
    slab_a_d = [None] + [
        nc.declare_dram_parameter(f"slab{s}a", [P, DT * int(Cs[s]) + W1B],
                                  f8, isOutput=False)
        for s in range(1, SLOTS)]
    slab_b_d = [None] + [
        nc.declare_dram_parameter(f"slab{s}b", [P, W2B], f8, isOutput=False)
        for s in range(1, SLOTS)]
    if has_b1:
        b1_d = nc.declare_dram_parameter("b1", [P, SLOTS * HT], f32, isOutput=False)
    # token-major output for slots 0..SLOTS-2: one dense per-partition line
    # per slot.  The last slot gets a dt-major tensor of its own so its
    # dt-pair stores are dense too (a [.., C, dt0:dt1] slice of a token-major
    # tensor would be a 4-byte-strided descriptor explosion).
    CT_HEAD = int(sum(Cs[:-1]))
    u_d = nc.declare_dram_parameter("u", [P, CT_HEAD, DT], bf16, isOutput=True)
    u2_d = nc.declare_dram_parameter("u2", [P, DT, int(Cs[-1])], bf16, isOutput=True)

    with tile.TileContext(nc) as tc:
        # PSUM budget (8 banks of 2KB/partition): hpsum 3x2 + upsum 2x1 = 8
        with tc.tile_pool(name="consts", bufs=1) as consts, \
             tc.tile_pool(name="hpool", bufs=2) as hpool, \
             tc.tile_pool(name="hpsum", bufs=3, space="PSUM") as hpsum, \
             tc.tile_pool(name="upsum", bufs=2, space="PSUM") as upsum, \
             tc.tile_pool(name="usb", bufs=3) as usb:

            slabA_sb = consts.tile(list(slabA_d.shape), f8, tag="slabA")
            slabB_sb = consts.tile(list(slabB_d.shape), f8, tag="slabB")
            slabC_sb = consts.tile(list(slabC_d.shape), f8, tag="slabC")
            slab_a_sb = [None] + [consts.tile(list(slab_a_d[s].shape), f8,
                                              tag=f"slab{s}a", name=f"slab_a{s}")
                                  for s in range(1, SLOTS)]
            slab_b_sb = [None] + [consts.tile(list(slab_b_d[s].shape), f8,
                                              tag=f"slab{s}b", name=f"slab_b{s}")
                                  for s in range(1, SLOTS)]
            if has_b1:
                b1_sb = consts.tile([P, SLOTS * HT], f32, tag="b1")
            zero_sb = consts.tile([P, HB, CMAX], f32, tag="zero")
            dummy_sb = consts.tile([P, 512], f8, tag="dummy")
            dscr_sb = consts.tile([P, 4], f32, tag="dscr")
            # gpsimd is the earliest-ready data engine after the framework
            # preamble; it seeds the warm-up operand so the PE can start
            # dummy matmuls ~1us sooner than a DVE memset would allow.
            nc.gpsimd.memset(dummy_sb[:, :], 0.0)

            def ztv(s):
                C = int(Cs[s])
                t = slabA_sb if s == 0 else slab_a_sb[s]
                return t[:, :DT * C].rearrange("p (d c) -> p d c", d=DT)

            def w1v(s, ht, j):
                # one 256B SwInterleave stationary block, as [P, 2, 128]
                if s == 0:
                    if ht < W1_HEAD_HT:
                        t, base = slabA_sb, DT * C0
                    else:
                        t, base, ht = slabB_sb, 0, ht - W1_HEAD_HT
                else:
                    t, base = slab_a_sb[s], DT * int(Cs[s])
                a = base + (ht * (DT // 2) + j) * 2 * P
                return t[:, a:a + 2 * P].rearrange("p (k m) -> p k m", k=2)

            def w2v(s, dt, t_):
                if s == 0:
                    t, base = slabC_sb, 0
                else:
                    t, base = slab_b_sb[s], 0
                a = base + (dt * (HT // 2) + t_) * 2 * P
                return t[:, a:a + 2 * P].rearrange("p (k m) -> p k m", k=2)

            # ---- PE warm-up: dummy matmuls with no input dependencies keep
            # the PE's HAM activity monitor busy from the engine's very first
            # available cycle, bridging gap-free into the real stream so the
            # 2.4 GHz clock gate (needs ~3.4us of sustained activity) opens
            # shortly after the real matmuls begin.
            phd = hpsum.tile([P, 2, 512], f32, tag="ph")
            for _ in range(WARMUP_MM):
                nc.tensor.matmul(
                    phd[:, 0, :256],
                    dummy_sb[:, :256].rearrange("p (k m) -> p k m", k=2),
                    dummy_sb[:, :].rearrange("p (k m) -> p k m", k=2),
                    start=True, stop=True, perf_mode=DR,
                )
            # tiny read of the warm PSUM so the matmuls can't be elided
            nc.vector.tensor_copy(dscr_sb[:, :], phd[:, 0, :4])

            # ---- resident loads: ALL on the Sync HWDGE ring, in exact
            # consumption order.  HWDGE executes one engine's transfers in
            # FIFO order, so each transfer gets the full SDMA bandwidth and
            # later slots can never starve the critical slot-0 path.
            # (measured: the Scalar HWDGE ring is slower for this first
            # transfer than queueing it first on Sync, so everything rides
            # the Sync ring)
            nc.sync.dma_start(slabA_sb[:, :], slabA_d[:, :])
            nc.sync.dma_start(slabB_sb[:, :], slabB_d[:, :])
            if has_b1:
                nc.sync.dma_start(b1_sb[:, :], b1_d[:, :])
            nc.sync.dma_start(slabC_sb[:, :], slabC_d[:, :])
            for s in range(1, SLOTS):
                nc.sync.dma_start(slab_a_sb[s][:, :], slab_a_d[s][:, :])
                nc.sync.dma_start(slab_b_sb[s][:, :], slab_b_d[s][:, :])

            # zeros for the DVE relu (max vs 0) path
            nc.vector.memset(zero_sb[:, :, :], 0.0)

            # ---- compute
            h_tiles, u_tiles = {}, {}

            def emit_l1_chunk(s, hb):
                # layer 1: h^T[ht] = relu(W1^T z^T + b1), 2 interleaved-fp8
                # matmuls (256-deep contraction each) per output tile.  Three
                # h-tiles share one 2-bank PSUM tile (3*C*4B <= 4KB) so the
                # relu drains them in a single op, alternating DVE / ACT
                # (GPSIMD can't read PSUM): fewer, bigger drains keep the
                # engines off the PE's critical path.
                C = int(Cs[s])
                if s not in h_tiles:
                    h_tiles[s] = hpool.tile([P, HT, C], f8, tag="h",
                                            name=f"h_{s}")
                h_sb = h_tiles[s]
                zt_ap = ztv(s)
                hts = range(HB * hb, min(HB * hb + HB, HT))
                n = len(hts)
                ph = hpsum.tile([P, HB, 512], f32, tag="ph")
                for i, ht in enumerate(hts):
                    for j in range(DT // 2):
                        nc.tensor.matmul(
                            ph[:, i, :C],
                            w1v(s, ht, j),
                            zt_ap[:, 2 * j:2 * j + 2, :],
                            start=(j == 0),
                            stop=(j == DT // 2 - 1),
                            perf_mode=DR,
                        )
                if has_b1:
                    # general path: ACT relu with per-partition bias
                    for i, ht in enumerate(hts):
                        nc.scalar.activation(
                            h_sb[:, ht, :], ph[:, i, :C], Relu,
                            bias=b1_sb[:, s * HT + ht: s * HT + ht + 1],
                        )
                elif hb % 2 == 0:
                    # DVE TT: relu(x) = max(x, 0) vs a zeros tile
                    nc.vector.tensor_max(
                        h_sb[:, HB * hb:HB * hb + n, :],
                        ph[:, :n, :C],
                        zero_sb[:, :n, :C])
                else:
                    nc.scalar.activation(
                        h_sb[:, HB * hb:HB * hb + n, :], ph[:, :n, :C], Relu)

            def emit_l2_chunk(s, dt):
                # layer 2: u^T[dt] = sum_ht W2[ht,dt]^T h^T[ht], 8 matmuls
                # per output tile.  Early slots stage token-major [C, DT];
                # the last slot stages dt-major so each drained dt-pair ships
                # immediately as a dense transfer.
                C = int(Cs[s])
                off = int(offs[s])
                last = s == SLOTS - 1
                if s not in u_tiles:
                    u_tiles[s] = usb.tile([P, DT, C] if last else [P, C, DT],
                                          bf16, tag="u2" if last else "u",
                                          name=f"u_{s}")
                u_sb = u_tiles[s]
                h_sb = h_tiles[s]
                pu = upsum.tile([P, CMAX], f32, tag="pu")
                for t in range(HT // 2):
                    nc.tensor.matmul(
                        pu[:, :C],
                        w2v(s, dt, t),
                        h_sb[:, 2 * t:2 * t + 2, :],
                        start=(t == 0),
                        stop=(t == HT // 2 - 1),
                        perf_mode=DR,
                    )
                u_ap = u_sb[:, dt, :] if last else u_sb[:, :, dt]
                if dt % 2 == 1:
                    nc.scalar.activation(u_ap, pu[:, :C], Copy)
                else:
                    nc.vector.tensor_copy(u_ap, pu[:, :C])
                if last and dt % 2 == 1:
                    # sync ring is idle by now; issuing there overlaps
                    # the store with the scalar engine's next ACT copy
                    nc.sync.dma_start(
                        u2_d[:, dt - 1:dt + 1, :],
                        u_sb[:, dt - 1:dt + 1, :])
                if not last and dt == DT - 1:
                    # earlier slots: one dense store on the Sync ring, queued
                    # BEHIND all weight slabs (FIFO) so the ~1MB of output
                    # writes never steal fabric bandwidth from the weight
                    # stream; they drain late when the ring is idle.
                    nc.sync.dma_start(u_d[:, off:off + C, :], u_sb[:, :, :])

            # Emission order: slot 0's L1 runs straight (supply-paced; the
            # drains hide under the incoming weight stream).  After that,
            # slot s-1's four L2 chains are spread evenly through slot s's
            # eight L1 chains: during an L1 burst the DVE/ACT relu drains
            # (~0.8us per 2-htile) can't keep up with the 0.51us PSUM fill
            # rate, so without the mix the PE stalls on PSUM-buffer reuse
            # while both engines idle during the following L2 burst.
            for hp in range(HT // 2):
                emit_l1_chunk(0, hp)
            for s in range(1, SLOTS):
                emit_l2_chunk(s - 1, 0)
                emit_l1_chunk(s, 0)
                emit_l1_chunk(s, 1)
                emit_l2_chunk(s - 1, 1)
                emit_l1_chunk(s, 2)
                emit_l1_chunk(s, 3)
                emit_l2_chunk(s - 1, 2)
                emit_l1_chunk(s, 4)
                emit_l1_chunk(s, 5)
                emit_l2_chunk(s - 1, 3)
                emit_l1_chunk(s, 6)
                emit_l1_chunk(s, 7)
            for dt in range(DT):
                emit_l2_chunk(SLOTS - 1, dt)
    nc.compile()
    return nc


def _get_nc(Cs, has_b1):
    key = (tuple(int(c) for c in Cs), bool(has_b1))
    if key not in _nc_cache:
        _nc_cache[key] = _build_nc(key[0], key[1])
    return _nc_cache[key]


def kernel(inp, ln_g, ln_b, wg_group, wg_inner, W1, b1, W2, b2, gln_g, gln_b):
    global LAST_RESULTS
    import jax
    import jax.numpy as jnp
    import ml_dtypes

    inp = np.asarray(inp)
    in_dtype = inp.dtype
    bf = ml_dtypes.bfloat16
    f8 = ml_dtypes.float8_e4m3

    # ---- 1. routing on host (bit-exact replica of the reference gates)
    gi, gsc, z, eis, escs = _routing(inp, ln_g, ln_b, wg_group, wg_inner)

    # token lists per (g, e)
    tok_lists, scale_lists = {}, {}
    for g in range(G):
        in_g = (gi == g).any(axis=1)
        S_g = np.nonzero(in_g)[0]
        ei, esc = eis[g], escs[g]
        for e in range(E):
            sel = ei[S_g] == e           # [|S_g|, EK]
            has = sel.any(axis=1)
            toks = S_g[has]
            w = (esc[S_g] * sel).sum(axis=1)[has]
            tok_lists[(g, e)] = toks
            scale_lists[(g, e)] = w.astype(np.float32)

    # ---- 2. balanced assignment of the 32 pairs to (core, slot)
    pairs = [(g, e) for g in range(G) for e in range(E)]
    pairs.sort(key=lambda p: -len(tok_lists[p]))
    assign = {}           # (core, slot) -> (g, e)
    Cs = []
    for s in range(SLOTS):
        rank = pairs[s * NCORES:(s + 1) * NCORES]
        Cs.append(max(CAP_GRAN, _round_up(max(len(tok_lists[p]) for p in rank), CAP_GRAN)))
        for c, p in enumerate(rank):
            assign[(c, s)] = p
    CT = int(sum(Cs))
    offs = np.concatenate([[0], np.cumsum(Cs)]).astype(int)

    # ---- 3. build per-core input maps
    W1n = np.asarray(W1, np.float32)
    W2n = np.asarray(W2, np.float32)
    b1n = np.asarray(b1, np.float32)
    b2n = np.asarray(b2, np.float32)
    z_f8 = z.astype(f8)
    has_b1 = bool(np.any(b1n))

    def _swi(W, n_in_tiles, n_out_tiles):
        # [K, M] weight -> the PE SwInterleave stationary layout
        # [ki, out_tile, pair, 256] with columns [A127 B127 .. A0 B0]
        # (pair-interleaved, out-column-reversed).
        Wv = W.astype(f8).reshape(n_in_tiles, P, n_out_tiles, P)  # [q, ki, ot, m]
        Wp = Wv.reshape(n_in_tiles // 2, 2, P, n_out_tiles, P)    # [pair, ab, ki, ot, m]
        Wr = Wp[..., ::-1]                                        # reverse m
        # -> [ki, ot, pair, m, ab] -> interleave (m, ab) into 256
        return np.ascontiguousarray(Wr.transpose(2, 3, 0, 4, 1)).reshape(
            P, n_out_tiles, n_in_tiles // 2, 2 * P)

    W1_HEAD_HT = 6
    in_maps = []
    for c in range(NCORES):
        im = {}
        if has_b1:
            b1_np = np.empty((P, SLOTS * HT), np.float32)
            b1_v = b1_np.reshape(P, SLOTS, HT)
            im["b1"] = b1_np
        for s in range(SLOTS):
            g, e = assign[(c, s)]
            toks = tok_lists[(g, e)]
            n = len(toks)
            # z^T tile (dt, p, c) -> [p, dt, c], one contiguous block per slot
            zt_np = np.zeros((P, DT, int(Cs[s])), f8)
            zt_np[:, :, :n] = z_f8[toks].T.reshape(DT, P, n).transpose(1, 0, 2)
            # [P, HT, DT//2, 256] / [P, DT, HT//2, 256] SwInterleave blocks
            w1_np = _swi(W1n[g, e], DT, HT).reshape(P, -1)
            w2_np = _swi(W2n[g, e], HT, DT).reshape(P, -1)
            zflat = zt_np.reshape(P, -1)
            w1_head_b = W1_HEAD_HT * (DT // 2) * 2 * P
            if s == 0:
                im["slabA"] = np.ascontiguousarray(
                    np.concatenate([zflat, w1_np[:, :w1_head_b]], axis=1))
                im["slabB"] = np.ascontiguousarray(w1_np[:, w1_head_b:])
                im["slabC"] = np.ascontiguousarray(w2_np)
            else:
                im[f"slab{s}a"] = np.ascontiguousarray(
                    np.concatenate([zflat, w1_np], axis=1))
                im[f"slab{s}b"] = np.ascontiguousarray(w2_np)
            if has_b1:
                b1_v[:, s, :] = b1n[g, e].reshape(HT, P).T
        in_maps.append(im)

    # ---- 4. compile + run on the 8 NeuronCores
    _ensure_ntff_hook()
    from concourse.bass_utils import run_bass_kernel_spmd

    nc = _get_nc(Cs, has_b1)
    res = run_bass_kernel_spmd(
        nc, in_maps, core_ids=list(range(NCORES)),
        trace=bool(int(os.environ.get("KERNEL_TRACE", "0"))),
    )
    LAST_RESULTS = res

    # ---- 5. host combine
    moe = np.zeros((G, N, D), np.float32)
    for c in range(NCORES):
        # u [p, CT_HEAD, dt] token-major + u2 [p, dt, C_last] dt-major
        # -> u^T[d, c] with d = dt*P + p -> [CT, D]
        u_head = (
            np.asarray(res.results[c]["u"], np.float32)
            .transpose(2, 0, 1).reshape(D, -1).T
        )
        u_last = (
            np.asarray(res.results[c]["u2"], np.float32)
            .transpose(1, 0, 2).reshape(D, -1).T
        )
        u = np.concatenate([u_head, u_last], axis=0)
        for s in range(SLOTS):
            g, e = assign[(c, s)]
            toks = tok_lists[(g, e)]
            n = len(toks)
            w = scale_lists[(g, e)]
            contrib = u[offs[s]:offs[s] + n] * w[:, None] + w[:, None] * b2n[g, e][None, :]
            np.add.at(moe[g], toks, contrib)

    cpu = jax.devices("cpu")[0]
    with jax.default_device(cpu):
        zj = jnp.asarray(z)
        gi_j = jnp.asarray(gi)
        gsc_j = jnp.asarray(gsc)
        gw_dense = jnp.sum(
            jax.nn.one_hot(gi_j, G, dtype=jnp.float32) * gsc_j[..., None], axis=-2
        )  # [N, G]
        out = jnp.zeros((N, D), jnp.float32)
        gg = jnp.asarray(np.asarray(gln_g, np.float32))
        gb = jnp.asarray(np.asarray(gln_b, np.float32))
        for g in range(G):
            t = zj + jnp.asarray(moe[g])
            m = jnp.mean(t, axis=-1, keepdims=True)
            tc_ = t - m
            v = jnp.mean(tc_ * tc_, axis=-1, keepdims=True)
            y = tc_ * jax.lax.rsqrt(v + EPS) * gg[g] + gb[g]
            out = out + gw_dense[:, g:g + 1] * y
        result = np.asarray(out).reshape(B, T, D) + np.asarray(inp, np.float32)

    return result.astype(in_dtype)
